# revision 20
# baseline (speedup 1.0000x reference)
"""Trainium2 Bass kernel for nn_Attention_22539988369511.

Dense transformer attention block (B=4, N=2048, C=1024, H=16, hd=64),
sharded over 8 NeuronCores with tensor parallelism over heads (2 heads
per core), AllToAll to re-shard from heads to tokens before the output
projection, host concatenation of per-core token slices.

Math notes (validated against the jax reference in a numpy mock):
 - x is fed pre-transposed as x^T [C, B*N] so every matmul contracts on
   the partition dim with no on-chip transposes.
 - q/k_norm_w are folded into the RoPE cos/sin tables on the host
   (legal since per-token RMS scaling commutes with rotation).
 - RMS factors r = sqrt(1/(sumsq + 64*eps)) omit the x8; the softmax
   scale folds it: q carries r_q (K=1 ones-outer-product broadcast
   matmul + multiply), while r_k rides the exp activation's
   per-partition scale AP after a K=1 transpose matmul (rhs=8.0 also
   folds the missing 8*8/8 scale).
 - Softmax skips max-subtraction: rms-normed scores are bounded, exp
   stays in fp32 range.
 - P@V appends a ones column to V so the softmax denominator falls out
   of the same matmul (M=65).
 - Pipeline is interleaved per batch: {qkv chunk, norm chunk} x2 then
   attention for that batch, so PE/ACT/DVE overlap across phases.
"""
import os
import sys

import numpy as np
import ml_dtypes

for _p in ("/opt/trn_rl_repo", "/root/.axon_site/_ro/trn_rl_repo"):
    if os.path.isdir(_p) and _p not in sys.path:
        sys.path.append(_p)

import concourse.bass as bass
import concourse.mybir as mybir
from concourse import bacc, tile
from concourse.bass_utils import run_bass_kernel_spmd

BF16 = ml_dtypes.bfloat16
F32 = mybir.dt.float32
BF = mybir.dt.bfloat16

NCORE = 8
B, N, C, H, HD = 4, 2048, 1024, 16, 64
T = B * N                 # 8192 tokens
HL = H // NCORE           # 2 heads per core
QKCH = HL * HD            # 128 q (or k) channels per core
TSLICE = T // NCORE       # 1024 tokens per core for the output projection
EPS_ROW = 64.0 * 1e-6     # eps folded into the sumsq matvec via an extra row
TOKC = 1024               # qkv/norm token chunk
QC = 1024                 # attention q chunk
KT = N // 128             # k tiles per batch (16)

_BUILD_CACHE = {}


def _build():
    if "nc" in _BUILD_CACHE:
        return _BUILD_CACHE["nc"]
    nc = bacc.Bacc(None, target_bir_lowering=False, debug=True)

    xT_d = nc.declare_dram_parameter("xT", [C, T], BF, isOutput=False)
    wqkvT_d = nc.declare_dram_parameter("wqkvT", [C, 3 * QKCH], BF, isOutput=False)
    wpT_d = nc.declare_dram_parameter("wpT", [C, C], BF, isOutput=False)
    bp_d = nc.declare_dram_parameter("bp", [1, C], BF, isOutput=False)
    cosq_d = nc.declare_dram_parameter("cosq", [HD, N], BF, isOutput=False)
    sinq_d = nc.declare_dram_parameter("sinq", [HD, N], BF, isOutput=False)
    cosk_d = nc.declare_dram_parameter("cosk", [HD, N], BF, isOutput=False)
    sink_d = nc.declare_dram_parameter("sink", [HD, N], BF, isOutput=False)
    out_d = nc.declare_dram_parameter("out", [TSLICE, C], mybir.dt.float32, isOutput=True)

    a2a_in = [nc.dram_tensor(f"a2a_in{hl}", [NCORE, HD, TSLICE], BF)
              for hl in range(HL)]
    a2a_out = [nc.dram_tensor(f"a2a_out{hl}", [NCORE, HD, TSLICE], BF)
               for hl in range(HL)]

    h2 = HD // 2

    with tile.TileContext(nc) as tc:
        with (
            tc.tile_pool(name="persist", bufs=1) as pp,
            tc.tile_pool(name="xt", bufs=2) as xtp,
            tc.tile_pool(name="nrm", bufs=2) as nrm,
            tc.tile_pool(name="nrm4", bufs=4) as nrm4,
            tc.tile_pool(name="att", bufs=3) as att,
            tc.tile_pool(name="attn1", bufs=2) as attn1,
            tc.tile_pool(name="yp", bufs=2) as yp,
            tc.tile_pool(name="pbig", bufs=2, space="PSUM") as pbig,
            tc.tile_pool(name="pacc", bufs=1, space="PSUM") as pacc,
            tc.tile_pool(name="psml", bufs=2, space="PSUM") as psml,
        ):
            # ---- resident tiles ----
            w_sb = pp.tile([128, 8, 3 * QKCH], BF)      # wqkvT, c-tiled
            bp_sb = pp.tile([1, C], BF)
            rope_sb = pp.tile([HD, 4, N], BF)           # cosq|sinq|cosk|sink
            qstore = pp.tile([QKCH, T], BF)
            kstore = pp.tile([QKCH, T], BF)
            vstore = pp.tile([128, T // 128, 2 * (HD + 1)], BF)
            shard = pp.tile([QKCH, T], BF)              # normalized out^T shard
            rkcol = pp.tile([128, HL, T // 128], F32)   # 8*r_k, column layout
            ones65 = pp.tile([HD + 1, 1], BF)
            ones1_64 = pp.tile([1, HD], BF)
            ones1_128 = pp.tile([1, 128], BF)
            eights1 = pp.tile([1, 1], BF)
            sqA = pp.tile([HD + 1, TOKC], BF)           # manual double buffer
            sqB = pp.tile([HD + 1, TOKC], BF)

            for c in range(8):
                nc.sync.dma_start(w_sb[:, c, :], wqkvT_d[128 * c:128 * (c + 1), :])
            nc.sync.dma_start(bp_sb[:], bp_d[:])
            for i, td in enumerate((cosq_d, sinq_d, cosk_d, sink_d)):
                nc.sync.dma_start(rope_sb[:, i, :], td[:])
            nc.vector.memset(ones65[:], 1.0)
            nc.vector.memset(ones1_64[:], 1.0)
            nc.vector.memset(ones1_128[:], 1.0)
            nc.vector.memset(eights1[:], 8.0)
            nc.vector.memset(vstore[:, :, HD:HD + 1], 1.0)
            nc.vector.memset(vstore[:, :, 2 * HD + 1:2 * HD + 2], 1.0)
            nc.vector.memset(sqA[HD:HD + 1, :], EPS_ROW)
            nc.vector.memset(sqB[HD:HD + 1, :], EPS_ROW)
            sqs = (sqA, sqB)

            # chain ACT instructions in emission order (sync=False) so the
            # scheduler keeps Ln/Exp bursts clustered -> few table loads
            acts = []

            def _act(inst):
                acts.append(inst)
                return inst

            def qkv_chunk(ti):
                tok0 = ti * TOKC
                xt = xtp.tile([128, 8, TOKC], BF, tag="xt")
                for c in range(8):
                    nc.sync.dma_start(
                        xt[:, c, :], xT_d[128 * c:128 * (c + 1), tok0:tok0 + TOKC])
                for m, store in ((0, qstore), (1, kstore)):
                    for t5 in range(TOKC // 512):
                        ps = psml.tile([128, 512], F32, tag="psml")
                        for c in range(8):
                            nc.tensor.matmul(
                                ps[:],
                                w_sb[:, c, m * QKCH:(m + 1) * QKCH],
                                xt[:, c, t5 * 512:(t5 + 1) * 512],
                                start=(c == 0), stop=(c == 7))
                        nc.vector.tensor_copy(
                            store[:, tok0 + t5 * 512:tok0 + (t5 + 1) * 512], ps[:])
                for t1 in range(TOKC // 128):
                    ps = psml.tile([128, 512], F32, tag="psml")
                    for c in range(8):
                        nc.tensor.matmul(
                            ps[:, 0:128],
                            xt[:, c, t1 * 128:(t1 + 1) * 128],
                            w_sb[:, c, 2 * QKCH:3 * QKCH],
                            start=(c == 0), stop=(c == 7))
                    g = (tok0 // 128) + t1
                    nc.vector.tensor_copy(
                        vstore[:, g, :].rearrange(
                            "p (a b) -> p a b", b=HD + 1)[:, :, 0:HD],
                        ps[:, 0:128].rearrange("p (a b) -> p a b", b=HD))

            def norm_chunk(ti):
                tok0 = ti * TOKC
                n0 = tok0 % N
                for hl in range(HL):
                    for m, store in ((0, qstore), (1, kstore)):
                        sl = store[HD * hl:HD * (hl + 1), tok0:tok0 + TOKC]
                        # base-partition-0 working copy (DVE two-SB-input ops
                        # require equal input base partitions)
                        qt = nrm.tile([HD, TOKC], BF, tag="qt")
                        nc.vector.tensor_copy(qt[:], sl)
                        sq = sqs[(hl * 2 + m) % 2]
                        nc.vector.tensor_mul(sq[0:HD, :], qt[:], qt[:])
                        rr = nrm.tile([1, TOKC], F32, tag="rr")
                        for ch in range(TOKC // 512):
                            ps = psml.tile([128, 512], F32, tag="psml")
                            nc.tensor.matmul(
                                ps[0:1, :], ones65[:],
                                sq[:, ch * 512:(ch + 1) * 512],
                                start=True, stop=True)
                            with nc.allow_low_precision(reason="rms scale"):
                                nc.vector.reciprocal_approx_fast(
                                    rr[:, ch * 512:(ch + 1) * 512], ps[0:1, :])
                        rr2 = nrm.tile([1, TOKC], BF, tag="rr2")
                        _act(nc.scalar.activation(
                            rr2[:], rr[:], mybir.ActivationFunctionType.Sqrt))
                        # rope
                        qrot = nrm.tile([HD, TOKC], BF, tag="qrot")
                        nc.vector.tensor_copy(qrot[0:h2, :], qt[h2:HD, :])
                        nc.vector.tensor_copy(qrot[h2:HD, :], qt[0:h2, :])
                        cw = rope_sb[:, 2 * m, n0:n0 + TOKC]
                        sw = rope_sb[:, 2 * m + 1, n0:n0 + TOKC]
                        tms = nrm.tile([HD, TOKC], BF, tag="tms")
                        nc.vector.tensor_mul(qt[:], qt[:], cw)
                        nc.vector.tensor_mul(tms[:], qrot[:], sw)
                        if m == 0:
                            # q: apply r via K=1 broadcast matmul + multiply
                            nc.vector.tensor_add(qt[:], qt[:], tms[:])
                            for ch in range(TOKC // 512):
                                psb = psml.tile([128, 512], F32, tag="psml")
                                nc.tensor.matmul(
                                    psb[0:HD, :], ones1_64[:],
                                    rr2[:, ch * 512:(ch + 1) * 512],
                                    start=True, stop=True)
                                nc.vector.tensor_mul(
                                    sl[:, ch * 512:(ch + 1) * 512],
                                    qt[:, ch * 512:(ch + 1) * 512], psb[0:HD, :])
                        else:
                            # k: r_k rides the exp scale; transpose 8*r_k into
                            # column layout via K=1 matmuls
                            nc.vector.tensor_add(sl, qt[:], tms[:])
                            pst = psml.tile([128, 512], F32, tag="psml")
                            for g in range(TOKC // 128):
                                nc.tensor.matmul(
                                    pst[:, g:g + 1],
                                    rr2[:, g * 128:(g + 1) * 128],
                                    eights1[:],
                                    start=True, stop=True)
                            nc.vector.tensor_copy(
                                rkcol[:, hl, tok0 // 128:tok0 // 128 + TOKC // 128],
                                pst[:, 0:TOKC // 128])

            def attention(hl, b):
                boff = b * N
                for qc in range(N // QC):
                    qoff = boff + qc * QC
                    pv = pacc.tile([HD + 1, QC], F32, tag="pacc")
                    for kt in range(KT):
                        koff = boff + kt * 128
                        sps = pbig.tile([128, QC], F32, tag="pbig")
                        for qh in range(QC // 512):
                            nc.tensor.matmul(
                                sps[:, qh * 512:(qh + 1) * 512],
                                kstore[HD * hl:HD * (hl + 1), koff:koff + 128],
                                qstore[HD * hl:HD * (hl + 1),
                                       qoff + qh * 512:qoff + (qh + 1) * 512],
                                start=True, stop=True)
                        pt = att.tile([128, QC], BF, tag="pt")
                        _act(nc.scalar.activation(
                            pt[:], sps[:], mybir.ActivationFunctionType.Exp,
                            scale=rkcol[:, hl, koff // 128:koff // 128 + 1]))
                        for qh in range(QC // 512):
                            nc.tensor.matmul(
                                pv[:, qh * 512:(qh + 1) * 512],
                                vstore[:, koff // 128,
                                       (HD + 1) * hl:(HD + 1) * (hl + 1)],
                                pt[:, qh * 512:(qh + 1) * 512],
                                start=(kt == 0), stop=(kt == KT - 1))
                    # custom-DVE recip mishandles base-partition-64 inputs;
                    # stage the denominator row at base 0 first
                    den0 = attn1.tile([1, QC], F32, tag="den0")
                    nc.vector.tensor_copy(den0[:], pv[HD:HD + 1, :])
                    drec = attn1.tile([1, QC], F32, tag="drec")
                    with nc.allow_low_precision(reason="softmax denom"):
                        nc.vector.reciprocal_approx_fast(drec[:], den0[:])
                    drecb = attn1.tile([1, QC], BF, tag="drecb")
                    nc.vector.tensor_copy(drecb[:], drec[:])
                    pvs = attn1.tile([HD, QC], BF, tag="pvs")
                    nc.vector.tensor_copy(pvs[:], pv[0:HD, :])
                    for q5 in range(QC // 512):
                        dbc = psml.tile([128, 512], F32, tag="psml")
                        nc.tensor.matmul(
                            dbc[0:HD, :], ones1_64[:],
                            drecb[:, q5 * 512:(q5 + 1) * 512],
                            start=True, stop=True)
                        nc.vector.tensor_mul(
                            shard[HD * hl:HD * (hl + 1),
                                  qoff + q5 * 512:qoff + (q5 + 1) * 512],
                            pvs[:, q5 * 512:(q5 + 1) * 512], dbc[0:HD, :])

            def reshard(hl):
                # AllToAll this head-half: heads -> token slices
                for j in range(NCORE):
                    nc.sync.dma_start(
                        a2a_in[hl][j],
                        shard[HD * hl:HD * (hl + 1), TSLICE * j:TSLICE * (j + 1)])
                nc.gpsimd.collective_compute(
                    "AllToAll",
                    mybir.AluOpType.bypass,
                    replica_groups=[list(range(NCORE))],
                    ins=[a2a_in[hl][:]],
                    outs=[a2a_out[hl][:]],
                )

            # ---- phased pipeline: qkv, norm, then attention ----
            for ti in range(T // TOKC):
                qkv_chunk(ti)
            for ti in range(T // TOKC):
                norm_chunk(ti)
            for b in range(B):
                attention(0, b)
            reshard(0)
            # second-head attention overlaps the first AllToAll
            for b in range(B):
                attention(1, b)
            reshard(1)

            # ---- output projection on this core's token slice ----
            # gat12 rows 0:64 <- head-half 0, rows 64:128 <- head-half 1, so
            # the K=64 half-contraction matmuls for half 0 can start while
            # the second AllToAll is still in flight.
            wp_sb = xtp.tile([128, 8, C], BF, tag="xt")
            for c in range(8):
                nc.sync.dma_start(wp_sb[:, c, :], wpT_d[128 * c:128 * (c + 1), :])
            gat = xtp.tile([128, 8, TSLICE], BF, tag="xt")
            for c in range(8):
                nc.sync.dma_start(gat[0:HD, c, :], a2a_out[0][c])
                nc.sync.dma_start(gat[HD:128, c, :], a2a_out[1][c])
            for t1 in range(TSLICE // 128):
                for d5 in range(C // 512):
                    ps = psml.tile([128, 512], F32, tag="psml")
                    for c in range(8):
                        nc.tensor.matmul(
                            ps[:],
                            gat[:, c, t1 * 128:(t1 + 1) * 128],
                            wp_sb[:, c, d5 * 512:(d5 + 1) * 512],
                            start=(c == 0), stop=False)
                    nc.tensor.matmul(
                        ps[:], ones1_128[:], bp_sb[:, d5 * 512:(d5 + 1) * 512],
                        start=False, stop=True)
                    ysb = yp.tile([128, 512], F32, tag="ysb")
                    nc.vector.tensor_copy(ysb[:], ps[:])
                    nc.sync.dma_start(
                        out_d[t1 * 128:(t1 + 1) * 128, d5 * 512:(d5 + 1) * 512],
                        ysb[:])

        for a, b2 in zip(acts, acts[1:]):
            tile.add_dep_helper(b2.ins, a.ins, sync=False, reason="act table cluster")

    nc.compile()
    _BUILD_CACHE["nc"] = nc
    return nc


def _host_prep(x, rope_cos, rope_sin, w_qkv, w_proj, b_proj, q_norm_w, k_norm_w):
    x = np.asarray(x, np.float32)
    xT = np.ascontiguousarray(x.reshape(T, C).T).astype(BF16)
    cosT = np.asarray(rope_cos, np.float32)[0, 0].T          # [hd, N]
    sinT = np.asarray(rope_sin, np.float32)[0, 0].T

    def fold(w):
        w = np.asarray(w, np.float32)
        cw = (cosT * w[:, None]).astype(BF16)
        sw = np.empty_like(sinT)
        sw[:32] = -sinT[:32] * w[32:64, None]
        sw[32:] = sinT[32:] * w[0:32, None]
        return cw, sw.astype(BF16)

    cosq, sinq = fold(q_norm_w)
    cosk, sink = fold(k_norm_w)
    wpT = np.ascontiguousarray(np.asarray(w_proj, np.float32).T).astype(BF16)
    bp = np.asarray(b_proj, np.float32).reshape(1, C).astype(BF16)
    w_qkv = np.asarray(w_qkv, np.float32)

    in_maps = []
    for r in range(NCORE):
        wq = w_qkv[QKCH * r:QKCH * (r + 1), :].T
        wk = w_qkv[C + QKCH * r:C + QKCH * (r + 1), :].T
        wv = w_qkv[2 * C + QKCH * r:2 * C + QKCH * (r + 1), :].T
        wqkvT = np.ascontiguousarray(
            np.concatenate([wq, wk, wv], axis=1)).astype(BF16)
        in_maps.append({
            "xT": xT, "wqkvT": wqkvT, "wpT": wpT, "bp": bp,
            "cosq": cosq, "sinq": sinq, "cosk": cosk, "sink": sink,
        })
    return in_maps


def _run(in_maps, trace=False, **kwargs):
    nc = _build()
    return run_bass_kernel_spmd(
        nc, in_maps, core_ids=list(range(NCORE)), trace=trace, **kwargs)


def kernel(**inputs):
    in_maps = _host_prep(**inputs)
    res = _run(in_maps)
    y = np.concatenate(
        [np.asarray(res.results[r]["out"], np.float32) for r in range(NCORE)],
        axis=0)
    return y.reshape(B, N, C)


# revision 22
# speedup vs baseline: 1.0599x; 1.0599x over previous
"""Trainium2 Bass kernel for nn_Attention_22539988369511.

Dense transformer attention block (B=4, N=2048, C=1024, H=16, hd=64),
sharded over 8 NeuronCores with tensor parallelism over heads (2 heads
per core), AllToAll to re-shard from heads to tokens before the output
projection, host concatenation of per-core token slices.

Math notes (validated against the jax reference in a numpy mock):
 - x is fed pre-transposed as x^T [C, B*N] so every matmul contracts on
   the partition dim with no on-chip transposes.
 - q/k_norm_w are folded into the RoPE cos/sin tables on the host
   (legal since per-token RMS scaling commutes with rotation).
 - RMS factors r = sqrt(1/(sumsq + 64*eps)) omit the x8; the softmax
   scale folds it: q carries r_q (K=1 ones-outer-product broadcast
   matmul + multiply), while r_k rides the exp activation's
   per-partition scale AP after a K=1 transpose matmul (rhs=8.0 also
   folds the missing 8*8/8 scale).
 - Softmax skips max-subtraction: rms-normed scores are bounded, exp
   stays in fp32 range.
 - P@V appends a ones column to V so the softmax denominator falls out
   of the same matmul (M=65).
 - Pipeline is interleaved per batch: {qkv chunk, norm chunk} x2 then
   attention for that batch, so PE/ACT/DVE overlap across phases.
"""
import os
import sys

import numpy as np
import ml_dtypes

for _p in ("/opt/trn_rl_repo", "/root/.axon_site/_ro/trn_rl_repo"):
    if os.path.isdir(_p) and _p not in sys.path:
        sys.path.append(_p)

import concourse.bass as bass
import concourse.mybir as mybir
from concourse import bacc, tile
from concourse.bass_utils import run_bass_kernel_spmd

BF16 = ml_dtypes.bfloat16
F32 = mybir.dt.float32
BF = mybir.dt.bfloat16

NCORE = 8
B, N, C, H, HD = 4, 2048, 1024, 16, 64
T = B * N                 # 8192 tokens
HL = H // NCORE           # 2 heads per core
QKCH = HL * HD            # 128 q (or k) channels per core
TSLICE = T // NCORE       # 1024 tokens per core for the output projection
EPS_ROW = 64.0 * 1e-6     # eps folded into the sumsq matvec via an extra row
TOKC = 1024               # qkv/norm token chunk
QC = 1024                 # attention q chunk
KT = N // 128             # k tiles per batch (16)

_BUILD_CACHE = {}


def _build():
    if "nc" in _BUILD_CACHE:
        return _BUILD_CACHE["nc"]
    nc = bacc.Bacc(None, target_bir_lowering=False, debug=True)

    xT_d = nc.declare_dram_parameter("xT", [C, T], BF, isOutput=False)
    wqkvT_d = nc.declare_dram_parameter("wqkvT", [C, 3 * QKCH], BF, isOutput=False)
    wpT_d = nc.declare_dram_parameter("wpT", [C, C], BF, isOutput=False)
    bp_d = nc.declare_dram_parameter("bp", [1, C], BF, isOutput=False)
    cosq_d = nc.declare_dram_parameter("cosq", [HD, N], BF, isOutput=False)
    sinq_d = nc.declare_dram_parameter("sinq", [HD, N], BF, isOutput=False)
    cosk_d = nc.declare_dram_parameter("cosk", [HD, N], BF, isOutput=False)
    sink_d = nc.declare_dram_parameter("sink", [HD, N], BF, isOutput=False)
    out_d = nc.declare_dram_parameter("out", [TSLICE, C], mybir.dt.float32, isOutput=True)

    a2a_in = [nc.dram_tensor(f"a2a_in{hl}", [NCORE, HD, TSLICE], BF)
              for hl in range(HL)]
    a2a_out = [nc.dram_tensor(f"a2a_out{hl}", [NCORE, HD, TSLICE], BF)
               for hl in range(HL)]

    h2 = HD // 2

    with tile.TileContext(nc) as tc:
        with (
            tc.tile_pool(name="persist", bufs=1) as pp,
            tc.tile_pool(name="xt", bufs=3) as xtp,
            tc.tile_pool(name="nrm", bufs=2) as nrm,
            tc.tile_pool(name="nrm1", bufs=1) as nrm1,
            tc.tile_pool(name="att", bufs=3) as att,
            tc.tile_pool(name="attn1", bufs=2) as attn1,
            tc.tile_pool(name="yp", bufs=2) as yp,
            tc.tile_pool(name="pbig", bufs=2, space="PSUM") as pbig,
            tc.tile_pool(name="pacc", bufs=1, space="PSUM") as pacc,
            tc.tile_pool(name="psml", bufs=2, space="PSUM") as psml,
        ):
            # ---- resident tiles ----
            w_sb = pp.tile([128, 8, 3 * QKCH], BF)      # wqkvT, c-tiled
            bp_sb = pp.tile([1, C], BF)
            # rope tables duplicated on partitions 64:128 so hl=1 slices can
            # be used in-place (DVE needs equal input base partitions)
            rope_sb = pp.tile([128, 4, N], BF)          # cosq|sinq|cosk|sink
            qstore = pp.tile([QKCH, T], BF)
            kstore = pp.tile([QKCH, T], BF)
            # partition-swapped copies: the score matmuls alternate the PE
            # array row-halves per k-tile so LDWEIGHTS overlaps MATMUL
            q2store = pp.tile([QKCH, T], BF)
            k2store = pp.tile([QKCH, T], BF)
            vstore = pp.tile([128, T // 128, 2 * (HD + 1)], BF)
            shard = pp.tile([QKCH, T], BF)              # normalized out^T shard
            rkcol = pp.tile([128, HL, T // 128], F32)   # 8*r_k, column layout
            ones65 = pp.tile([HD + 1, 1], BF)
            ones1_64 = pp.tile([1, HD], BF)
            ones1_128 = pp.tile([1, 128], BF)
            eights1 = pp.tile([1, 1], BF)
            sqA = pp.tile([HD + 1, TOKC], BF)           # manual double buffer
            sqB = pp.tile([HD + 1, TOKC], BF)

            for c in range(8):
                nc.sync.dma_start(w_sb[:, c, :], wqkvT_d[128 * c:128 * (c + 1), :])
            nc.sync.dma_start(bp_sb[:], bp_d[:])
            for i, td in enumerate((cosq_d, sinq_d, cosk_d, sink_d)):
                nc.sync.dma_start(rope_sb[0:HD, i, :], td[:])
                nc.sync.dma_start(rope_sb[HD:128, i, :], td[:])
            nc.vector.memset(ones65[:], 1.0)
            nc.vector.memset(ones1_64[:], 1.0)
            nc.vector.memset(ones1_128[:], 1.0)
            nc.vector.memset(eights1[:], 8.0)
            nc.vector.memset(vstore[:, :, HD:HD + 1], 1.0)
            nc.vector.memset(vstore[:, :, 2 * HD + 1:2 * HD + 2], 1.0)
            nc.vector.memset(sqA[HD:HD + 1, :], EPS_ROW)
            nc.vector.memset(sqB[HD:HD + 1, :], EPS_ROW)
            sqs = (sqA, sqB)

            # chain ACT instructions in emission order (sync=False) so the
            # scheduler keeps Sqrt/Exp bursts clustered -> few table loads
            acts = []

            def _act(inst):
                acts.append(inst)
                return inst

            def qkv_chunk(ti):
                tok0 = ti * TOKC
                for t5 in range(TOKC // 512):
                    tk0 = tok0 + t5 * 512
                    xt = xtp.tile([128, 8, 512], BF, tag="xt")
                    for c in range(8):
                        nc.sync.dma_start(
                            xt[:, c, :], xT_d[128 * c:128 * (c + 1), tk0:tk0 + 512])
                    for m, store in ((0, qstore), (1, kstore)):
                        ps = psml.tile([128, 512], F32, tag="psml")
                        for c in range(8):
                            nc.tensor.matmul(
                                ps[:],
                                w_sb[:, c, m * QKCH:(m + 1) * QKCH],
                                xt[:, c, :],
                                start=(c == 0), stop=(c == 7))
                        nc.vector.tensor_copy(store[:, tk0:tk0 + 512], ps[:])
                    for t1 in range(4):
                        ps = psml.tile([128, 512], F32, tag="psml")
                        for c in range(8):
                            nc.tensor.matmul(
                                ps[:, 0:128],
                                xt[:, c, t1 * 128:(t1 + 1) * 128],
                                w_sb[:, c, 2 * QKCH:3 * QKCH],
                                start=(c == 0), stop=(c == 7))
                        g = (tk0 // 128) + t1
                        nc.vector.tensor_copy(
                            vstore[:, g, :].rearrange(
                                "p (a b) -> p a b", b=HD + 1)[:, :, 0:HD],
                            ps[:, 0:128].rearrange("p (a b) -> p a b", b=HD))

            def norm_chunk(ti):
                tok0 = ti * TOKC
                n0 = tok0 % N
                for hl in range(HL):
                    r0 = HD * hl
                    for m, store in ((0, qstore), (1, kstore)):
                        sl = store[r0:r0 + HD, tok0:tok0 + TOKC]
                        sq = sqs[(hl * 2 + m) % 2]
                        nc.vector.tensor_mul(sq[0:HD, :], sl, sl)
                        rr = nrm1.tile([1, TOKC], F32, tag="rr")
                        for ch in range(TOKC // 512):
                            ps = psml.tile([128, 512], F32, tag="psml")
                            nc.tensor.matmul(
                                ps[0:1, :], ones65[:],
                                sq[:, ch * 512:(ch + 1) * 512],
                                start=True, stop=True)
                            with nc.allow_low_precision(reason="rms scale"):
                                nc.vector.reciprocal_approx_fast(
                                    rr[:, ch * 512:(ch + 1) * 512], ps[0:1, :])
                        rr2 = nrm.tile([1, TOKC], BF, tag="rr2")
                        _act(nc.scalar.activation(
                            rr2[:], rr[:], mybir.ActivationFunctionType.Sqrt))
                        # rope, in place at this hl's native base partition
                        qrot = nrm.tile([128, TOKC], BF, tag="qrot")
                        nc.vector.tensor_copy(qrot[r0:r0 + h2, :],
                                              store[r0 + h2:r0 + HD, tok0:tok0 + TOKC])
                        nc.vector.tensor_copy(qrot[r0 + h2:r0 + HD, :],
                                              store[r0:r0 + h2, tok0:tok0 + TOKC])
                        cw = rope_sb[r0:r0 + HD, 2 * m, n0:n0 + TOKC]
                        sw = rope_sb[r0:r0 + HD, 2 * m + 1, n0:n0 + TOKC]
                        tms = nrm.tile([128, TOKC], BF, tag="tms")
                        nc.vector.tensor_mul(sl, sl, cw)
                        nc.vector.tensor_mul(tms[r0:r0 + HD, :],
                                             qrot[r0:r0 + HD, :], sw)
                        nc.vector.tensor_add(sl, sl, tms[r0:r0 + HD, :])
                        if m == 0:
                            # q: apply r via K=1 broadcast matmul + multiply
                            for ch in range(TOKC // 512):
                                psb = psml.tile([128, 512], F32, tag="psml")
                                nc.tensor.matmul(
                                    psb[r0:r0 + HD, :], ones1_64[:],
                                    rr2[:, ch * 512:(ch + 1) * 512],
                                    start=True, stop=True,
                                    tile_position=(0, r0))
                                nc.vector.tensor_mul(
                                    sl[:, ch * 512:(ch + 1) * 512],
                                    sl[:, ch * 512:(ch + 1) * 512],
                                    psb[r0:r0 + HD, :])
                            nc.vector.tensor_copy(
                                q2store[HD - r0:2 * HD - r0, tok0:tok0 + TOKC], sl)
                        else:
                            # k: r_k rides the exp scale; transpose 8*r_k into
                            # column layout via K=1 matmuls
                            pst = psml.tile([128, 512], F32, tag="psml")
                            for g in range(TOKC // 128):
                                nc.tensor.matmul(
                                    pst[:, g:g + 1],
                                    rr2[:, g * 128:(g + 1) * 128],
                                    eights1[:],
                                    start=True, stop=True)
                            nc.vector.tensor_copy(
                                rkcol[:, hl, tok0 // 128:tok0 // 128 + TOKC // 128],
                                pst[:, 0:TOKC // 128])
                            nc.vector.tensor_copy(
                                k2store[HD - r0:2 * HD - r0, tok0:tok0 + TOKC], sl)

            def attention(hl, b):
                r0 = HD * hl
                r1 = HD - r0  # swapped-store base for odd k-tiles
                boff = b * N
                for qc in range(N // QC):
                    qoff = boff + qc * QC
                    pv = pacc.tile([HD + 1, QC], F32, tag="pacc")
                    for kt in range(KT):
                        koff = boff + kt * 128
                        sps = pbig.tile([128, QC], F32, tag="pbig")
                        if kt % 2 == 0:
                            ks, qs, base = kstore, qstore, r0
                        else:
                            ks, qs, base = k2store, q2store, r1
                        for qh in range(QC // 512):
                            nc.tensor.matmul(
                                sps[:, qh * 512:(qh + 1) * 512],
                                ks[base:base + HD, koff:koff + 128],
                                qs[base:base + HD,
                                   qoff + qh * 512:qoff + (qh + 1) * 512],
                                start=True, stop=True)
                        pt = att.tile([128, QC], BF, tag="pt")
                        _act(nc.scalar.activation(
                            pt[:], sps[:], mybir.ActivationFunctionType.Exp,
                            scale=rkcol[:, hl, koff // 128:koff // 128 + 1]))
                        for qh in range(QC // 512):
                            nc.tensor.matmul(
                                pv[:, qh * 512:(qh + 1) * 512],
                                vstore[:, koff // 128,
                                       (HD + 1) * hl:(HD + 1) * (hl + 1)],
                                pt[:, qh * 512:(qh + 1) * 512],
                                start=(kt == 0), stop=(kt == KT - 1))
                    # custom-DVE recip mishandles base-partition-64 inputs;
                    # stage the denominator row at base 0 first
                    den0 = attn1.tile([1, QC], F32, tag="den0")
                    nc.vector.tensor_copy(den0[:], pv[HD:HD + 1, :])
                    drec = attn1.tile([1, QC], F32, tag="drec")
                    with nc.allow_low_precision(reason="softmax denom"):
                        nc.vector.reciprocal_approx_fast(drec[:], den0[:])
                    drecb = attn1.tile([1, QC], BF, tag="drecb")
                    nc.vector.tensor_copy(drecb[:], drec[:])
                    pvs = attn1.tile([HD, QC], BF, tag="pvs")
                    nc.vector.tensor_copy(pvs[:], pv[0:HD, :])
                    for q5 in range(QC // 512):
                        dbc = psml.tile([128, 512], F32, tag="psml")
                        nc.tensor.matmul(
                            dbc[0:HD, :], ones1_64[:],
                            drecb[:, q5 * 512:(q5 + 1) * 512],
                            start=True, stop=True)
                        nc.vector.tensor_mul(
                            shard[r0:r0 + HD,
                                  qoff + q5 * 512:qoff + (q5 + 1) * 512],
                            pvs[:, q5 * 512:(q5 + 1) * 512], dbc[0:HD, :])

            def reshard(hl):
                # AllToAll this head-half: heads -> token slices
                for j in range(NCORE):
                    nc.sync.dma_start(
                        a2a_in[hl][j],
                        shard[HD * hl:HD * (hl + 1), TSLICE * j:TSLICE * (j + 1)])
                nc.gpsimd.collective_compute(
                    "AllToAll",
                    mybir.AluOpType.bypass,
                    replica_groups=[list(range(NCORE))],
                    ins=[a2a_in[hl][:]],
                    outs=[a2a_out[hl][:]],
                )

            # ---- interleaved pipeline: qkv/norm + first-head attention ----
            for b in range(B):
                for ti in (2 * b, 2 * b + 1):
                    qkv_chunk(ti)
                    norm_chunk(ti)
                attention(0, b)
            reshard(0)
            # second-head attention overlaps the first AllToAll
            for b in range(B):
                attention(1, b)
            reshard(1)

            # ---- output projection on this core's token slice ----
            wp1 = xtp.tile([128, 8, 512], BF, tag="xt")
            wp2 = xtp.tile([128, 8, 512], BF, tag="xt")
            for c in range(8):
                nc.sync.dma_start(wp1[:, c, :], wpT_d[128 * c:128 * (c + 1), 0:512])
                nc.sync.dma_start(wp2[:, c, :], wpT_d[128 * c:128 * (c + 1), 512:1024])
            wps = (wp1, wp2)
            for th in range(2):  # token halves of this core's slice
                gat = xtp.tile([128, 8, 512], BF, tag="xt")
                for c in range(8):
                    nc.sync.dma_start(gat[0:HD, c, :],
                                      a2a_out[0][c, :, th * 512:(th + 1) * 512])
                    nc.sync.dma_start(gat[HD:128, c, :],
                                      a2a_out[1][c, :, th * 512:(th + 1) * 512])
                for t1 in range(4):
                    for d5 in range(2):
                        ps = psml.tile([128, 512], F32, tag="psml")
                        for c in range(8):
                            nc.tensor.matmul(
                                ps[:],
                                gat[:, c, t1 * 128:(t1 + 1) * 128],
                                wps[d5][:, c, :],
                                start=(c == 0), stop=False)
                        nc.tensor.matmul(
                            ps[:], ones1_128[:], bp_sb[:, d5 * 512:(d5 + 1) * 512],
                            start=False, stop=True)
                        ysb = yp.tile([128, 512], F32, tag="ysb")
                        nc.vector.tensor_copy(ysb[:], ps[:])
                        nc.sync.dma_start(
                            out_d[th * 512 + t1 * 128:th * 512 + (t1 + 1) * 128,
                                  d5 * 512:(d5 + 1) * 512],
                            ysb[:])

        for a, b2 in zip(acts, acts[1:]):
            tile.add_dep_helper(b2.ins, a.ins, sync=False, reason="act table cluster")

    nc.compile()
    _BUILD_CACHE["nc"] = nc
    return nc


def _host_prep(x, rope_cos, rope_sin, w_qkv, w_proj, b_proj, q_norm_w, k_norm_w):
    x = np.asarray(x, np.float32)
    xT = np.ascontiguousarray(x.reshape(T, C).T).astype(BF16)
    cosT = np.asarray(rope_cos, np.float32)[0, 0].T          # [hd, N]
    sinT = np.asarray(rope_sin, np.float32)[0, 0].T

    def fold(w):
        w = np.asarray(w, np.float32)
        cw = (cosT * w[:, None]).astype(BF16)
        sw = np.empty_like(sinT)
        sw[:32] = -sinT[:32] * w[32:64, None]
        sw[32:] = sinT[32:] * w[0:32, None]
        return cw, sw.astype(BF16)

    cosq, sinq = fold(q_norm_w)
    cosk, sink = fold(k_norm_w)
    wpT = np.ascontiguousarray(np.asarray(w_proj, np.float32).T).astype(BF16)
    bp = np.asarray(b_proj, np.float32).reshape(1, C).astype(BF16)
    w_qkv = np.asarray(w_qkv, np.float32)

    in_maps = []
    for r in range(NCORE):
        wq = w_qkv[QKCH * r:QKCH * (r + 1), :].T
        wk = w_qkv[C + QKCH * r:C + QKCH * (r + 1), :].T
        wv = w_qkv[2 * C + QKCH * r:2 * C + QKCH * (r + 1), :].T
        wqkvT = np.ascontiguousarray(
            np.concatenate([wq, wk, wv], axis=1)).astype(BF16)
        in_maps.append({
            "xT": xT, "wqkvT": wqkvT, "wpT": wpT, "bp": bp,
            "cosq": cosq, "sinq": sinq, "cosk": cosk, "sink": sink,
        })
    return in_maps


def _run(in_maps, trace=False, **kwargs):
    nc = _build()
    return run_bass_kernel_spmd(
        nc, in_maps, core_ids=list(range(NCORE)), trace=trace, **kwargs)


def kernel(**inputs):
    in_maps = _host_prep(**inputs)
    res = _run(in_maps)
    y = np.concatenate(
        [np.asarray(res.results[r]["out"], np.float32) for r in range(NCORE)],
        axis=0)
    return y.reshape(B, N, C)


# revision 23
# speedup vs baseline: 1.1689x; 1.1029x over previous
"""Trainium2 Bass kernel for nn_Attention_22539988369511.

Dense transformer attention block (B=4, N=2048, C=1024, H=16, hd=64),
sharded over 8 NeuronCores with tensor parallelism over heads (2 heads
per core), AllToAll to re-shard from heads to tokens before the output
projection, host concatenation of per-core token slices.

Math notes (validated against the jax reference in a numpy mock):
 - x is fed pre-transposed as x^T [C, B*N] so every matmul contracts on
   the partition dim with no on-chip transposes.
 - q/k_norm_w are folded into the RoPE cos/sin tables on the host
   (legal since per-token RMS scaling commutes with rotation).
 - RMS factors r = sqrt(1/(sumsq + 64*eps)) omit the x8; the softmax
   scale folds it: q carries r_q (K=1 ones-outer-product broadcast
   matmul + multiply), while r_k rides the exp activation's
   per-partition scale AP after a K=1 transpose matmul (rhs=8.0 also
   folds the missing 8*8/8 scale).
 - Softmax skips max-subtraction: rms-normed scores are bounded, exp
   stays in fp32 range.
 - P@V appends a ones column to V so the softmax denominator falls out
   of the same matmul (M=65).
 - Pipeline is interleaved per batch: {qkv chunk, norm chunk} x2 then
   attention for that batch, so PE/ACT/DVE overlap across phases.
"""
import os
import sys

import numpy as np
import ml_dtypes

for _p in ("/opt/trn_rl_repo", "/root/.axon_site/_ro/trn_rl_repo"):
    if os.path.isdir(_p) and _p not in sys.path:
        sys.path.append(_p)

import concourse.bass as bass
import concourse.mybir as mybir
from concourse import bacc, tile
from concourse.bass_utils import run_bass_kernel_spmd

BF16 = ml_dtypes.bfloat16
F32 = mybir.dt.float32
BF = mybir.dt.bfloat16

NCORE = 8
B, N, C, H, HD = 4, 2048, 1024, 16, 64
T = B * N                 # 8192 tokens
HL = H // NCORE           # 2 heads per core
QKCH = HL * HD            # 128 q (or k) channels per core
TSLICE = T // NCORE       # 1024 tokens per core for the output projection
EPS_ROW = 64.0 * 1e-6     # eps folded into the sumsq matvec via an extra row
TOKC = 1024               # qkv/norm token chunk
QC = 1024                 # attention q chunk
KT = N // 128             # k tiles per batch (16)

_BUILD_CACHE = {}


def _build():
    if "nc" in _BUILD_CACHE:
        return _BUILD_CACHE["nc"]
    nc = bacc.Bacc(None, target_bir_lowering=False, debug=True)

    xT_d = nc.declare_dram_parameter("xT", [C, T], BF, isOutput=False)
    wqkvT_d = nc.declare_dram_parameter("wqkvT", [C, 3 * QKCH], BF, isOutput=False)
    wpT_d = nc.declare_dram_parameter("wpT", [C, C], BF, isOutput=False)
    bp_d = nc.declare_dram_parameter("bp", [1, C], BF, isOutput=False)
    cosq_d = nc.declare_dram_parameter("cosq", [HD, N], BF, isOutput=False)
    sinq_d = nc.declare_dram_parameter("sinq", [HD, N], BF, isOutput=False)
    cosk_d = nc.declare_dram_parameter("cosk", [HD, N], BF, isOutput=False)
    sink_d = nc.declare_dram_parameter("sink", [HD, N], BF, isOutput=False)
    out_d = nc.declare_dram_parameter("out", [TSLICE, C], mybir.dt.float32, isOutput=True)

    a2a_in = [nc.dram_tensor(f"a2a_in{hl}", [NCORE, HD, TSLICE], BF)
              for hl in range(HL)]
    a2a_out = [nc.dram_tensor(f"a2a_out{hl}", [NCORE, HD, TSLICE], BF)
               for hl in range(HL)]

    h2 = HD // 2

    with tile.TileContext(nc) as tc:
        with (
            tc.tile_pool(name="persist", bufs=1) as pp,
            tc.tile_pool(name="xt", bufs=3) as xtp,
            tc.tile_pool(name="nrm", bufs=2) as nrm,
            tc.tile_pool(name="nrm1", bufs=1) as nrm1,
            tc.tile_pool(name="att", bufs=3) as att,
            tc.tile_pool(name="attn1", bufs=2) as attn1,
            tc.tile_pool(name="yp", bufs=2) as yp,
            tc.tile_pool(name="pbig", bufs=2, space="PSUM") as pbig,
            tc.tile_pool(name="pacc", bufs=1, space="PSUM") as pacc,
            tc.tile_pool(name="psml", bufs=2, space="PSUM") as psml,
        ):
            # ---- resident tiles ----
            w_sb = pp.tile([128, 8, 3 * QKCH], BF)      # wqkvT, c-tiled
            bp_sb = pp.tile([1, C], BF)
            # rope tables duplicated on partitions 64:128 so hl=1 slices can
            # be used in-place (DVE needs equal input base partitions)
            rope_sb = pp.tile([128, 4, N], BF)          # cosq|sinq|cosk|sink
            qstore = pp.tile([QKCH, T], BF)
            kstore = pp.tile([QKCH, T], BF)
            # partition-swapped copies: the score matmuls alternate the PE
            # array row-halves per k-tile so LDWEIGHTS overlaps MATMUL
            q2store = pp.tile([QKCH, T], BF)
            k2store = pp.tile([QKCH, T], BF)
            vstore = pp.tile([128, T // 128, 2 * (HD + 1)], BF)
            shard = pp.tile([QKCH, T], BF)              # normalized out^T shard
            rkcol = pp.tile([128, HL, T // 128], F32)   # 8*r_k, column layout
            ones128c = pp.tile([128, 1], BF)
            ones1_64 = pp.tile([1, HD], BF)
            ones1_128 = pp.tile([1, 128], BF)
            eights1 = pp.tile([1, 1], BF)
            sqA = pp.tile([128, TOKC], BF)              # manual double buffer
            sqB = pp.tile([128, TOKC], BF)

            for c in range(8):
                nc.sync.dma_start(w_sb[:, c, :], wqkvT_d[128 * c:128 * (c + 1), :])
            nc.sync.dma_start(bp_sb[:], bp_d[:])
            for i, td in enumerate((cosq_d, sinq_d, cosk_d, sink_d)):
                nc.sync.dma_start(rope_sb[0:HD, i, :], td[:])
                nc.sync.dma_start(rope_sb[HD:128, i, :], td[:])
            nc.vector.memset(ones128c[:], 1.0)
            nc.vector.memset(ones1_64[:], 1.0)
            nc.vector.memset(ones1_128[:], 1.0)
            nc.vector.memset(eights1[:], 8.0)
            nc.vector.memset(vstore[:, :, HD:HD + 1], 1.0)
            nc.vector.memset(vstore[:, :, 2 * HD + 1:2 * HD + 2], 1.0)
            sqs = (sqA, sqB)

            # chain ACT instructions in emission order (sync=False) so the
            # scheduler keeps Sqrt/Exp bursts clustered -> few table loads
            acts = []

            def _act(inst):
                acts.append(inst)
                return inst

            def qkv_chunk(ti):
                tok0 = ti * TOKC
                for t5 in range(TOKC // 512):
                    tk0 = tok0 + t5 * 512
                    xt = xtp.tile([128, 8, 512], BF, tag="xt")
                    for c in range(8):
                        nc.sync.dma_start(
                            xt[:, c, :], xT_d[128 * c:128 * (c + 1), tk0:tk0 + 512])
                    for m, store in ((0, qstore), (1, kstore)):
                        ps = psml.tile([128, 512], F32, tag="psml")
                        for c in range(8):
                            nc.tensor.matmul(
                                ps[:],
                                w_sb[:, c, m * QKCH:(m + 1) * QKCH],
                                xt[:, c, :],
                                start=(c == 0), stop=(c == 7))
                        nc.vector.tensor_copy(store[:, tk0:tk0 + 512], ps[:])
                    for t1 in range(4):
                        ps = psml.tile([128, 512], F32, tag="psml")
                        for c in range(8):
                            nc.tensor.matmul(
                                ps[:, 0:128],
                                xt[:, c, t1 * 128:(t1 + 1) * 128],
                                w_sb[:, c, 2 * QKCH:3 * QKCH],
                                start=(c == 0), stop=(c == 7))
                        g = (tk0 // 128) + t1
                        nc.vector.tensor_copy(
                            vstore[:, g, :].rearrange(
                                "p (a b) -> p a b", b=HD + 1)[:, :, 0:HD],
                            ps[:, 0:128].rearrange("p (a b) -> p a b", b=HD))

            def norm_chunk(ti):
                tok0 = ti * TOKC
                n0 = tok0 % N
                for m, store in ((0, qstore), (1, kstore)):
                    slf = store[:, tok0:tok0 + TOKC]     # both heads at once
                    sq2 = sqs[m]
                    nc.vector.tensor_mul(sq2[:], slf, slf)
                    rr2s = {}
                    for hl in range(HL):
                        r0 = HD * hl
                        rr = nrm1.tile([1, TOKC], F32, tag="rr")
                        for ch in range(TOKC // 512):
                            ps = psml.tile([128, 512], F32, tag="psml")
                            nc.tensor.matmul(
                                ps[0:1, :], ones128c[r0:r0 + HD, 0:1],
                                sq2[r0:r0 + HD, ch * 512:(ch + 1) * 512],
                                start=True, stop=True)
                            with nc.allow_low_precision(reason="rms scale"):
                                nc.vector.reciprocal_approx_fast(
                                    rr[:, ch * 512:(ch + 1) * 512], ps[0:1, :])
                        rr2 = nrm.tile([1, TOKC], BF, tag="rr2")
                        rr2s[hl] = rr2
                        _act(nc.scalar.activation(
                            rr2[:], rr[:], mybir.ActivationFunctionType.Sqrt))
                    # rope on all 128 partitions (tables duplicated per head)
                    qrot = nrm.tile([128, TOKC], BF, tag="qrot")
                    for r0 in (0, HD):
                        nc.vector.tensor_copy(
                            qrot[r0:r0 + h2, :],
                            store[r0 + h2:r0 + HD, tok0:tok0 + TOKC])
                        nc.vector.tensor_copy(
                            qrot[r0 + h2:r0 + HD, :],
                            store[r0:r0 + h2, tok0:tok0 + TOKC])
                    cw = rope_sb[:, 2 * m, n0:n0 + TOKC]
                    sw = rope_sb[:, 2 * m + 1, n0:n0 + TOKC]
                    tms = nrm.tile([128, TOKC], BF, tag="tms")
                    nc.vector.tensor_mul(slf, slf, cw)
                    nc.vector.tensor_mul(tms[:], qrot[:], sw)
                    nc.vector.tensor_add(slf, slf, tms[:])
                    if m == 0:
                        # q: apply r (both heads per op) via K=1 broadcasts
                        for ch in range(TOKC // 512):
                            psb = psml.tile([128, 512], F32, tag="psml")
                            nc.tensor.matmul(
                                psb[0:HD, :], ones1_64[:],
                                rr2s[0][:, ch * 512:(ch + 1) * 512],
                                start=True, stop=True, tile_position=(0, 0))
                            nc.tensor.matmul(
                                psb[HD:128, :], ones1_64[:],
                                rr2s[1][:, ch * 512:(ch + 1) * 512],
                                start=True, stop=True, tile_position=(0, HD))
                            nc.vector.tensor_mul(
                                slf[:, ch * 512:(ch + 1) * 512],
                                slf[:, ch * 512:(ch + 1) * 512], psb[:])
                        nc.vector.tensor_copy(
                            q2store[HD:128, tok0:tok0 + TOKC],
                            store[0:HD, tok0:tok0 + TOKC])
                        nc.vector.tensor_copy(
                            q2store[0:HD, tok0:tok0 + TOKC],
                            store[HD:128, tok0:tok0 + TOKC])
                    else:
                        # k: r_k rides the exp scale; transpose 8*r_k into
                        # column layout via K=1 matmuls
                        pst = psml.tile([128, 512], F32, tag="psml")
                        for hl in range(HL):
                            for g in range(TOKC // 128):
                                nc.tensor.matmul(
                                    pst[:, hl * 8 + g:hl * 8 + g + 1],
                                    rr2s[hl][:, g * 128:(g + 1) * 128],
                                    eights1[:],
                                    start=True, stop=True)
                        for hl in range(HL):
                            nc.vector.tensor_copy(
                                rkcol[:, hl, tok0 // 128:tok0 // 128 + TOKC // 128],
                                pst[:, hl * 8:hl * 8 + TOKC // 128])
                        nc.vector.tensor_copy(
                            k2store[HD:128, tok0:tok0 + TOKC],
                            store[0:HD, tok0:tok0 + TOKC])
                        nc.vector.tensor_copy(
                            k2store[0:HD, tok0:tok0 + TOKC],
                            store[HD:128, tok0:tok0 + TOKC])

            def attention(hl, b):
                r0 = HD * hl
                r1 = HD - r0  # swapped-store base for odd k-tiles
                boff = b * N
                for qc in range(N // QC):
                    qoff = boff + qc * QC
                    pv = pacc.tile([HD + 1, QC], F32, tag="pacc")
                    for kt in range(KT):
                        koff = boff + kt * 128
                        sps = pbig.tile([128, QC], F32, tag="pbig")
                        if kt % 2 == 0:
                            ks, qs, base = kstore, qstore, r0
                        else:
                            ks, qs, base = k2store, q2store, r1
                        for qh in range(QC // 512):
                            nc.tensor.matmul(
                                sps[:, qh * 512:(qh + 1) * 512],
                                ks[base:base + HD, koff:koff + 128],
                                qs[base:base + HD,
                                   qoff + qh * 512:qoff + (qh + 1) * 512],
                                start=True, stop=True)
                        pt = att.tile([128, QC], BF, tag="pt")
                        _act(nc.scalar.activation(
                            pt[:], sps[:], mybir.ActivationFunctionType.Exp,
                            scale=rkcol[:, hl, koff // 128:koff // 128 + 1]))
                        for qh in range(QC // 512):
                            nc.tensor.matmul(
                                pv[:, qh * 512:(qh + 1) * 512],
                                vstore[:, koff // 128,
                                       (HD + 1) * hl:(HD + 1) * (hl + 1)],
                                pt[:, qh * 512:(qh + 1) * 512],
                                start=(kt == 0), stop=(kt == KT - 1))
                    # custom-DVE recip mishandles base-partition-64 inputs;
                    # stage the denominator row at base 0 first
                    den0 = attn1.tile([1, QC], F32, tag="den0")
                    nc.vector.tensor_copy(den0[:], pv[HD:HD + 1, :])
                    drec = attn1.tile([1, QC], F32, tag="drec")
                    with nc.allow_low_precision(reason="softmax denom"):
                        nc.vector.reciprocal_approx_fast(drec[:], den0[:])
                    drecb = attn1.tile([1, QC], BF, tag="drecb")
                    nc.vector.tensor_copy(drecb[:], drec[:])
                    pvs = attn1.tile([HD, QC], BF, tag="pvs")
                    nc.vector.tensor_copy(pvs[:], pv[0:HD, :])
                    for q5 in range(QC // 512):
                        dbc = psml.tile([128, 512], F32, tag="psml")
                        nc.tensor.matmul(
                            dbc[0:HD, :], ones1_64[:],
                            drecb[:, q5 * 512:(q5 + 1) * 512],
                            start=True, stop=True)
                        nc.vector.tensor_mul(
                            shard[r0:r0 + HD,
                                  qoff + q5 * 512:qoff + (q5 + 1) * 512],
                            pvs[:, q5 * 512:(q5 + 1) * 512], dbc[0:HD, :])

            def reshard(hl):
                # AllToAll this head-half: heads -> token slices
                for j in range(NCORE):
                    nc.sync.dma_start(
                        a2a_in[hl][j],
                        shard[HD * hl:HD * (hl + 1), TSLICE * j:TSLICE * (j + 1)])
                nc.gpsimd.collective_compute(
                    "AllToAll",
                    mybir.AluOpType.bypass,
                    replica_groups=[list(range(NCORE))],
                    ins=[a2a_in[hl][:]],
                    outs=[a2a_out[hl][:]],
                )

            # ---- interleaved pipeline: qkv/norm + first-head attention ----
            for b in range(B):
                for ti in (2 * b, 2 * b + 1):
                    qkv_chunk(ti)
                    norm_chunk(ti)
                attention(0, b)
            reshard(0)
            # second-head attention overlaps the first AllToAll
            for b in range(B):
                attention(1, b)
            reshard(1)

            # ---- output projection on this core's token slice ----
            wp1 = xtp.tile([128, 8, 512], BF, tag="xt")
            wp2 = xtp.tile([128, 8, 512], BF, tag="xt")
            for c in range(8):
                nc.sync.dma_start(wp1[:, c, :], wpT_d[128 * c:128 * (c + 1), 0:512])
                nc.sync.dma_start(wp2[:, c, :], wpT_d[128 * c:128 * (c + 1), 512:1024])
            wps = (wp1, wp2)
            for th in range(2):  # token halves of this core's slice
                gat = xtp.tile([128, 8, 512], BF, tag="xt")
                for c in range(8):
                    nc.sync.dma_start(gat[0:HD, c, :],
                                      a2a_out[0][c, :, th * 512:(th + 1) * 512])
                    nc.sync.dma_start(gat[HD:128, c, :],
                                      a2a_out[1][c, :, th * 512:(th + 1) * 512])
                for t1 in range(4):
                    for d5 in range(2):
                        ps = psml.tile([128, 512], F32, tag="psml")
                        for c in range(8):
                            nc.tensor.matmul(
                                ps[:],
                                gat[:, c, t1 * 128:(t1 + 1) * 128],
                                wps[d5][:, c, :],
                                start=(c == 0), stop=False)
                        nc.tensor.matmul(
                            ps[:], ones1_128[:], bp_sb[:, d5 * 512:(d5 + 1) * 512],
                            start=False, stop=True)
                        ysb = yp.tile([128, 512], F32, tag="ysb")
                        nc.vector.tensor_copy(ysb[:], ps[:])
                        nc.sync.dma_start(
                            out_d[th * 512 + t1 * 128:th * 512 + (t1 + 1) * 128,
                                  d5 * 512:(d5 + 1) * 512],
                            ysb[:])

        for a, b2 in zip(acts, acts[1:]):
            tile.add_dep_helper(b2.ins, a.ins, sync=False, reason="act table cluster")

    nc.compile()
    _BUILD_CACHE["nc"] = nc
    return nc


def _host_prep(x, rope_cos, rope_sin, w_qkv, w_proj, b_proj, q_norm_w, k_norm_w):
    x = np.asarray(x, np.float32)
    xT = np.ascontiguousarray(x.reshape(T, C).T).astype(BF16)
    cosT = np.asarray(rope_cos, np.float32)[0, 0].T          # [hd, N]
    sinT = np.asarray(rope_sin, np.float32)[0, 0].T

    def fold(w):
        w = np.asarray(w, np.float32)
        cw = (cosT * w[:, None]).astype(BF16)
        sw = np.empty_like(sinT)
        sw[:32] = -sinT[:32] * w[32:64, None]
        sw[32:] = sinT[32:] * w[0:32, None]
        return cw, sw.astype(BF16)

    cosq, sinq = fold(q_norm_w)
    cosk, sink = fold(k_norm_w)
    wpT = np.ascontiguousarray(np.asarray(w_proj, np.float32).T).astype(BF16)
    bp = np.asarray(b_proj, np.float32).reshape(1, C).astype(BF16)
    w_qkv = np.asarray(w_qkv, np.float32)

    in_maps = []
    for r in range(NCORE):
        wq = w_qkv[QKCH * r:QKCH * (r + 1), :].T
        wk = w_qkv[C + QKCH * r:C + QKCH * (r + 1), :].T
        wv = w_qkv[2 * C + QKCH * r:2 * C + QKCH * (r + 1), :].T
        wqkvT = np.ascontiguousarray(
            np.concatenate([wq, wk, wv], axis=1)).astype(BF16)
        in_maps.append({
            "xT": xT, "wqkvT": wqkvT, "wpT": wpT, "bp": bp,
            "cosq": cosq, "sinq": sinq, "cosk": cosk, "sink": sink,
        })
    return in_maps


def _run(in_maps, trace=False, **kwargs):
    nc = _build()
    return run_bass_kernel_spmd(
        nc, in_maps, core_ids=list(range(NCORE)), trace=trace, **kwargs)


def kernel(**inputs):
    in_maps = _host_prep(**inputs)
    res = _run(in_maps)
    y = np.concatenate(
        [np.asarray(res.results[r]["out"], np.float32) for r in range(NCORE)],
        axis=0)
    return y.reshape(B, N, C)
